# revision 9
# baseline (speedup 1.0000x reference)
"""Trainium2 Bass kernel for quantum-augmented MultiHeadAttention.

Math: the per-head "quantum layer" is affine (pre-matmul, Givens rotations,
post-matmul, residual), so it folds into a 64x64 matrix applied to each
head's slice of the QKV projections.  The device kernel is then a plain
multi-head attention:
    q = query @ wq_eff.T ; k = key @ wk_eff.T ; v = value @ wv.T
    out = softmax(q k^T / 8) v @ wo.T
Sharding: 8 cores = (4 batches) x (2 head-groups of 8 heads).  Each core
computes its head-group's attention and a row-sharded partial of the output
projection; the two partials per batch are summed on the host.

Device layout (per core):
    QT/KT [512=8*64, S] head-dim-major (computed as W @ X^T), V [S, 8*65]
    with a ones column per head (gives the softmax denominator for free).
    Scores are computed transposed (S.T[sk, sq]) so every matmul operand is
    naturally laid out; exp on the scalar engine (the kernel bottleneck);
    O^T accumulated over sk tiles in PSUM; softmax normalization via
    gpsimd partition_broadcast of the reciprocal denominators.
Matmul operands are bf16 (the only full-rate PE dtype on this toolchain);
accumulation is fp32 in PSUM.  Optionally the projection weights are split
into bf16 hi+lo pairs (split_w) to halve the projection rounding error --
the extra matmuls hide under the ACT-engine exp bottleneck.
"""

import sys

sys.path.insert(0, "/opt/trn_rl_repo")

import numpy as np
import ml_dtypes

BF16 = ml_dtypes.bfloat16
EMBED = 1024
HEADS = 16
HD = 64
NQ = 6
HPC = 8  # heads per core
DC = HPC * HD  # 512 head dims per core
N_CORES = 8
S = 2048
SPLIT_W = True  # hi/lo-split projection weights (better precision, PE-free)

_CACHE = {}


# ----------------------------------------------------------------- host math
def _rot_matrix(theta, phi):
    """64x64 matrix M with  res_out = res_in @ M  for the qubit rotations."""
    M = np.eye(HD, dtype=np.float64)
    idx = np.arange(HD)
    for i in range(NQ):
        c = np.cos(np.float64(theta[i]))
        s = np.sin(np.float64(theta[i]))
        cp = np.cos(np.float64(phi[i]))
        i0 = np.where(((idx >> i) & 1) == 0)[0]
        i1 = i0 + (1 << i)
        Mi = np.zeros((HD, HD), dtype=np.float64)
        Mi[i0, i0] = c
        Mi[i1, i0] = -s
        Mi[i0, i1] = s * cp
        Mi[i1, i1] = c * cp
        M = M @ Mi
    return M


def _quantum_fold(theta, phi, preW, preb, postW, postb):
    """quantum_layer(x) == x @ Weff + beff   (row-vector convention)."""
    M = _rot_matrix(theta, phi)
    core = preW.T.astype(np.float64) @ M @ postW.T.astype(np.float64)
    Weff = core + np.eye(HD)
    beff = preb.astype(np.float64) @ M @ postW.T.astype(np.float64) + postb
    return Weff, beff


def _fold_weights(wq, wk, q_fold, k_fold, scale_q):
    """Per-head fold of the quantum Weff into the projection weights."""
    Wq_eff = np.empty((EMBED, EMBED), dtype=np.float64)
    Wk_eff = np.empty((EMBED, EMBED), dtype=np.float64)
    for h in range(HEADS):
        sl = slice(h * HD, (h + 1) * HD)
        Wq_eff[:, sl] = wq[sl, :].astype(np.float64).T @ q_fold
        Wk_eff[:, sl] = wk[sl, :].astype(np.float64).T @ k_fold
    Wq_eff *= scale_q
    return Wq_eff.astype(np.float32), Wk_eff.astype(np.float32)


def _hi_lo(w):
    hi = w.astype(BF16)
    lo = (w - hi.astype(np.float32)).astype(BF16)
    return hi, lo


# ------------------------------------------------------------ device program
def build_program(seq_len=S, loop_n=None, split_w=SPLIT_W):
    """Build the per-core Bass program.  Returns a compiled Bacc.

    loop_n: if set, wrap the whole body in a For_i hardware loop (for
    timing).  The body is idempotent so repetition is safe.
    """
    import concourse.tile as tile
    from concourse import bacc, mybir
    from contextlib import ExitStack, nullcontext

    f32 = mybir.dt.float32
    bf16 = mybir.dt.bfloat16
    FN = mybir.ActivationFunctionType

    SQ = 512  # sq chunk (PSUM bank width in fp32)
    nsqc = seq_len // SQ
    nskt = seq_len // 128
    nmt = seq_len // 128
    nw = 2 if split_w else 1  # weight parts (hi, lo)

    nc = bacc.Bacc("TRN2", target_bir_lowering=False, debug=False,
                   num_devices=N_CORES)

    xq = nc.dram_tensor("xq_t", [EMBED, seq_len], bf16, kind="ExternalInput").ap()
    xk = nc.dram_tensor("xk_t", [EMBED, seq_len], bf16, kind="ExternalInput").ap()
    xv = nc.dram_tensor("xv_t", [EMBED, seq_len], bf16, kind="ExternalInput").ap()
    wqd = nc.dram_tensor("wq_t", [nw, EMBED, DC], bf16, kind="ExternalInput").ap()
    wkd = nc.dram_tensor("wk_t", [nw, EMBED, DC], bf16, kind="ExternalInput").ap()
    wvd = nc.dram_tensor("wv_t", [nw, EMBED, DC], bf16, kind="ExternalInput").ap()
    wod = nc.dram_tensor("wo_t", [DC, EMBED], bf16, kind="ExternalInput").ap()
    onesd = nc.dram_tensor("ones_d", [128, HPC], bf16, kind="ExternalInput").ap()
    outd = nc.dram_tensor("out", [seq_len, EMBED], f32, kind="ExternalOutput").ap()

    with tile.TileContext(nc) as tc, ExitStack() as top:
        qkv = top.enter_context(tc.tile_pool(name="qkv", bufs=1))
        qt = [qkv.tile([128, seq_len], bf16, name=f"qt{m}", tag=f"qt{m}")
              for m in range(4)]
        kt = [qkv.tile([128, seq_len], bf16, name=f"kt{m}", tag=f"kt{m}")
              for m in range(4)]
        vt = [qkv.tile([128, HPC * (HD + 1)], bf16, name=f"vt{i}", tag=f"vt{i}")
              for i in range(nskt)]

        loop = tc.For_i(0, loop_n, 1) if loop_n else nullcontext()
        with loop:
            # V ones columns (softmax denominator accumulators)
            for i in range(nskt):
                dst = vt[i][:].rearrange("p (h c) -> p h c", h=HPC)[:, :, HD]
                nc.sync.dma_start(dst, onesd[:, :])

            # ---------------- phase A: projections (v, k, q order) --------
            with tc.tile_pool(name="wp", bufs=1) as wp, \
                 tc.tile_pool(name="xp", bufs=2) as xp, \
                 tc.tile_pool(name="pp", bufs=4, space="PSUM") as pp:
                wsb = {}
                for name, dram in (("v", wvd), ("k", wkd), ("q", wqd)):
                    wsb[name] = [wp.tile([128, DC], bf16, name=f"w_{name}{k}",
                                         tag=f"w_{name}{k}")
                                 for k in range(8 * nw)]
                    for p in range(nw):
                        for k in range(8):
                            nc.sync.dma_start(
                                wsb[name][p * 8 + k][:],
                                dram[p, k * 128:(k + 1) * 128, :])

                for name, xd in (("v", xv), ("k", xk), ("q", xq)):
                    for sqc in range(nsqc):
                        xtiles = []
                        for k in range(8):
                            t = xp.tile([128, SQ], bf16, name=f"x{k}", tag=f"x{k}")
                            nc.sync.dma_start(
                                t[:], xd[k * 128:(k + 1) * 128,
                                         sqc * SQ:(sqc + 1) * SQ])
                            xtiles.append(t)
                        if name == "v":
                            # V[sq, d] : lhsT = X^T slice, rhs = Wv
                            for st4 in range(SQ // 128):
                                i = sqc * (SQ // 128) + st4
                                ps = pp.tile([128, DC], f32, name="ps_v", tag="ps")
                                for k in range(8):
                                    for p in range(nw):
                                        nc.tensor.matmul(
                                            ps[:],
                                            xtiles[k][:, st4 * 128:(st4 + 1) * 128],
                                            wsb["v"][p * 8 + k][:],
                                            start=(k == 0 and p == 0),
                                            stop=(k == 7 and p == nw - 1))
                                src = ps[:].rearrange("p (h c) -> p h c", h=HPC)
                                dst = vt[i][:].rearrange(
                                    "p (h c) -> p h c", h=HPC)[:, :, 0:HD]
                                nc.vector.tensor_copy(dst, src)
                        else:
                            # QT/KT [d, sq] : lhsT = W slice, rhs = X^T
                            dstblocks = qt if name == "q" else kt
                            for m in range(4):
                                ps = pp.tile([128, SQ], f32, name="ps_qk", tag="ps")
                                for k in range(8):
                                    for p in range(nw):
                                        nc.tensor.matmul(
                                            ps[:],
                                            wsb[name][p * 8 + k][:, m * 128:(m + 1) * 128],
                                            xtiles[k][:],
                                            start=(k == 0 and p == 0),
                                            stop=(k == 7 and p == nw - 1))
                                nc.vector.tensor_copy(
                                    dstblocks[m][:, sqc * SQ:(sqc + 1) * SQ],
                                    ps[:])

            # ---------------- phase B: attention --------------------------
            with tc.tile_pool(name="ot", bufs=1) as otp:
                ot = [otp.tile([128, seq_len], bf16, name=f"ot{m}", tag=f"ot{m}")
                      for m in range(4)]

                with tc.tile_pool(name="pt", bufs=3) as ptp, \
                     tc.tile_pool(name="rc", bufs=4) as rcp, \
                     tc.tile_pool(name="stp", bufs=2, space="PSUM") as stp, \
                     tc.tile_pool(name="op", bufs=4, space="PSUM") as opp:
                    for sqc in range(nsqc):
                        for hp in range(4):
                            o_ps = [opp.tile([128, SQ], f32, name=f"o_ps{j}",
                                             tag="o_ps") for j in range(2)]
                            for skt in range(nskt):
                                st = stp.tile([128, 2 * SQ], f32, name="st",
                                              tag="st")
                                pt = ptp.tile([128, 2 * SQ], bf16, name="pt",
                                              tag="pt")
                                for j in range(2):
                                    nc.tensor.matmul(
                                        st[:, j * SQ:(j + 1) * SQ],
                                        kt[hp][j * HD:(j + 1) * HD,
                                               skt * 128:(skt + 1) * 128],
                                        qt[hp][j * HD:(j + 1) * HD,
                                               sqc * SQ:(sqc + 1) * SQ],
                                        start=True, stop=True)
                                nc.scalar.activation(pt[:], st[:], FN.Exp)
                                for j in range(2):
                                    h = 2 * hp + j
                                    nc.tensor.matmul(
                                        o_ps[j][0:HD + 1, :],
                                        vt[skt][:, h * (HD + 1):(h + 1) * (HD + 1)],
                                        pt[:, j * SQ:(j + 1) * SQ],
                                        start=(skt == 0), stop=(skt == nskt - 1))
                            for j in range(2):
                                rc = rcp.tile([1, SQ], f32, name="rc", tag="rc")
                                nc.vector.reciprocal(rc[:], o_ps[j][HD:HD + 1, :])
                                bcs = rcp.tile([HD, SQ], f32, name="bcs", tag="bcs")
                                nc.gpsimd.partition_broadcast(bcs[:], rc[0:1, :])
                                nc.vector.tensor_mul(
                                    ot[hp][j * HD:(j + 1) * HD,
                                           sqc * SQ:(sqc + 1) * SQ],
                                    o_ps[j][0:HD, :], bcs[:])

                # ------------- phase C: output projection -----------------
                with tc.tile_pool(name="wo", bufs=1) as wop, \
                     tc.tile_pool(name="ob", bufs=2) as obp, \
                     tc.tile_pool(name="os", bufs=4, space="PSUM") as osp:
                    wo_sb = [wop.tile([128, EMBED], bf16, name=f"wo{k}",
                                      tag=f"wo{k}") for k in range(4)]
                    for k in range(4):
                        nc.sync.dma_start(wo_sb[k][:],
                                          wod[k * 128:(k + 1) * 128, :])
                    for mt in range(nmt):
                        ob = obp.tile([128, EMBED], f32, name="ob", tag="ob")
                        for nch in range(2):
                            ps = osp.tile([128, 512], f32, name="ps_o", tag="ps_o")
                            for kb in range(4):
                                nc.tensor.matmul(
                                    ps[:],
                                    ot[kb][:, mt * 128:(mt + 1) * 128],
                                    wo_sb[kb][:, nch * 512:(nch + 1) * 512],
                                    start=(kb == 0), stop=(kb == 3))
                            nc.vector.tensor_copy(
                                ob[:, nch * 512:(nch + 1) * 512], ps[:])
                        nc.sync.dma_start(outd[mt * 128:(mt + 1) * 128, :], ob[:])

    nc.compile()
    return nc


# ----------------------------------------------------------------- interface
def _prepare(inputs, seq_len, split_w=SPLIT_W):
    """Host-side fold + shard.  Returns (in_maps, bo)."""
    q_fold, q_beff = _quantum_fold(inputs["q_theta"], inputs["q_phi"],
                                   inputs["q_preW"], inputs["q_preb"],
                                   inputs["q_postW"], inputs["q_postb"])
    k_fold, k_beff = _quantum_fold(inputs["k_theta"], inputs["k_phi"],
                                   inputs["k_preW"], inputs["k_preb"],
                                   inputs["k_postW"], inputs["k_postb"])
    for b in (inputs["bq"], inputs["bk"], inputs["bv"], q_beff, k_beff):
        assert np.abs(np.asarray(b, dtype=np.float64)).max() == 0.0, \
            "nonzero bias path not implemented"

    scale_q = 1.0 / np.sqrt(np.float32(HD))
    wq_eff, wk_eff = _fold_weights(inputs["wq"], inputs["wk"],
                                   q_fold, k_fold, scale_q)
    wv_t = np.ascontiguousarray(inputs["wv"].T).astype(np.float32)
    wo = inputs["wo"]

    def wparts(w):  # [E, DC_slice] fp32 -> [nw, E, DC] bf16
        if split_w:
            hi, lo = _hi_lo(w)
            return np.stack([hi, lo])
        return w.astype(BF16)[None]

    B = inputs["query"].shape[0]
    xq_t = np.ascontiguousarray(inputs["query"].transpose(0, 2, 1)).astype(BF16)
    xk_t = np.ascontiguousarray(inputs["key"].transpose(0, 2, 1)).astype(BF16)
    xv_t = np.ascontiguousarray(inputs["value"].transpose(0, 2, 1)).astype(BF16)

    in_maps = []
    for c in range(N_CORES):
        b, hg = divmod(c, 2)
        b = b % B
        sl = slice(hg * DC, (hg + 1) * DC)
        in_maps.append({
            "xq_t": xq_t[b, :, :seq_len],
            "xk_t": xk_t[b, :, :seq_len],
            "xv_t": xv_t[b, :, :seq_len],
            "wq_t": wparts(wq_eff[:, sl]),
            "wk_t": wparts(wk_eff[:, sl]),
            "wv_t": wparts(wv_t[:, sl]),
            "wo_t": np.ascontiguousarray(wo[:, sl].T).astype(BF16),
            "ones_d": np.ones((128, HPC), dtype=BF16),
        })
    return in_maps, inputs["bo"]


def _install_hook_tracer():
    """Surface compile errors that the PJRT layer otherwise swallows."""
    if _CACHE.get("hook"):
        return
    _CACHE["hook"] = True
    try:
        from concourse import bass2jax
        bass2jax.install_neuronx_cc_hook()
        import libneuronxla
        orig = libneuronxla.neuronx_cc

        def wrapped(*args, **kwargs):
            try:
                return orig(*args, **kwargs)
            except Exception:
                import traceback
                traceback.print_exc()
                raise
        libneuronxla.neuronx_cc = wrapped
    except Exception:
        pass


def kernel(**inputs):
    from concourse.bass_utils import run_bass_kernel_spmd

    _install_hook_tracer()
    if "prog" not in _CACHE:
        _CACHE["prog"] = build_program(S)
    nc = _CACHE["prog"]

    in_maps, bo = _prepare(inputs, S)
    res = run_bass_kernel_spmd(nc, in_maps, core_ids=list(range(N_CORES)))

    B = inputs["query"].shape[0]
    out = np.empty((B, S, EMBED), dtype=np.float32)
    for b in range(B):
        out[b] = res.results[2 * b]["out"] + res.results[2 * b + 1]["out"]
    out += np.asarray(inputs["bo"]).reshape(1, 1, EMBED).astype(np.float32)
    return out
